# revision 43
# baseline (speedup 1.0000x reference)
"""Trainium2 Bass kernel: multi-head self-attention (B=2, T=2048, D=1024, H=16),
8-core SPMD. Accepts FULL inputs, returns the FULL output.

Sharding: data-parallel over batch (2) x tensor-parallel over heads (4 groups
of 4). Each core computes attention for its 4 heads of one batch plus its
partial output projection; the host sums the 4 partials per batch (plus the
bias terms, folded exactly). Matmuls run in bf16 on the PE (2x the fp32r
stream rate); accumulation is fp32 in PSUM and softmax denominators stay fp32.

Schedule: a short phase-B prefix (Q/K for head-pair 0 over the first T half,
V tiles 0-3) runs as soon as its DMAs land; everything else (remaining QKV
projection tiles, normalization, output projection) is interleaved into the
attention key-chunk stream, which is paced by the exp() activations on the
scalar engine. Causal-mask multiplies run on the otherwise-idle GpSimd engine
so the P-tile critical path never queues behind vector-engine copies.
"""
import sys
if '/opt/trn_rl_repo' not in sys.path:
    sys.path.insert(0, '/opt/trn_rl_repo')
import numpy as np
import ml_dtypes
import concourse.bass as bass
import concourse.mybir as mybir
from concourse import bacc
from concourse.tile import TileContext

F32 = mybir.dt.float32
F32R = mybir.dt.float32r
BF16 = mybir.dt.bfloat16
AL = mybir.AluOpType
EXP = mybir.ActivationFunctionType.Exp

T = 2048
DM = 1024
HPC = 4
D = 64
NQB = 4           # query blocks of 512
NKC = 16          # key chunks of 128
NDC = 8           # contraction chunks of 128 for projections
LAG = 3           # PV lags S/exp by this many key chunks


def build_nc():
    nc = bacc.Bacc("TRN2", target_bir_lowering=False, debug=True)

    xT = nc.dram_tensor("xT", [DM, T], BF16, kind="ExternalInput")
    wq = nc.dram_tensor("wq", [DM, 256], BF16, kind="ExternalInput")
    wk = nc.dram_tensor("wk", [DM, 256], BF16, kind="ExternalInput")
    wv = nc.dram_tensor("wv", [DM, 260], BF16, kind="ExternalInput")
    wp = nc.dram_tensor("wp", [2, 128, DM], BF16, kind="ExternalInput")
    msk = nc.dram_tensor("msk", [NQB, 128, 1024], BF16, kind="ExternalInput")
    y = nc.dram_tensor("y", [T, DM], BF16, kind="ExternalOutput")

    with nc.allow_low_precision("bf16 matmul pipeline"), TileContext(nc) as tc:
        from contextlib import ExitStack
        ctx = ExitStack()
        cp = ctx.enter_context(tc.tile_pool(name="const", bufs=1))
        wtp = ctx.enter_context(tc.tile_pool(name="wts", bufs=1))
        qkvp = ctx.enter_context(tc.tile_pool(name="qkv", bufs=1))
        xtp = ctx.enter_context(tc.tile_pool(name="xt", bufs=1))
        psS = ctx.enter_context(tc.tile_pool(name="psS", bufs=2, space="PSUM"))
        psO = ctx.enter_context(tc.tile_pool(name="psO", bufs=1, space="PSUM"))
        psX = ctx.enter_context(tc.tile_pool(name="psX", bufs=2, space="PSUM"))

        mask_t = [cp.tile([128, 1024], BF16, tag=f"m{i}", name=f"m{i}")
                  for i in range(NQB)]
        ones32 = cp.tile([128, 64], F32, tag="ones32", name="ones32")
        nc.vector.memset(ones32[:], 1.0)
        # preload the exp ACT table set while the input DMAs stream in
        warm = cp.tile([1, 8], F32, tag="warm", name="warm")
        nc.scalar.activation(warm[:], ones32[0:1, 0:8], EXP)
        ones_t = cp.tile([128, 64], BF16, tag="ones", name="ones")
        nc.vector.tensor_copy(ones_t[:], ones32[:])
        onesv = cp.tile([128, 4], F32, tag="onesv", name="onesv")
        nc.vector.memset(onesv[:], 1.0)
        # PE warm-up: dummy matmuls with no DMA deps flip the HAM clock
        # gate to 8/8 (2.4 GHz) before the real streams begin
        wmt = cp.tile([128, 512], BF16, tag="wmt", name="wmt")
        nc.vector.memset(wmt[:], 0.0)
        psW = psX.tile([128, 512], F32, tag="b", name="b")
        for i in range(20):
            # ~6us: bridges the input-DMA ramp so the PE never idles a
            # full MID window before the first projection group
            nc.tensor.matmul(psW[:], wmt[:, 0:128], wmt[:],
                             start=(i == 0), stop=(i == 19))

        # weights
        wq_t = [wtp.tile([128, 256], BF16, tag=f"wq{k}", name=f"wq{k}")
                for k in range(NDC)]
        wk_t = [wtp.tile([128, 256], BF16, tag=f"wk{k}", name=f"wk{k}")
                for k in range(NDC)]
        wv_t = [wtp.tile([128, 260], BF16, tag=f"wv{k}", name=f"wv{k}")
                for k in range(NDC)]
        wp_t = [wtp.tile([128, DM], BF16, tag=f"wp{j}", name=f"wp{j}")
                for j in range(2)]

        # persistent activations
        QT = [qkvp.tile([128, T], BF16, tag=f"qt{i}", name=f"qt{i}") for i in range(2)]
        KT = [qkvp.tile([128, T], BF16, tag=f"kt{i}", name=f"kt{i}") for i in range(2)]
        V = [qkvp.tile([128, 260], BF16, tag=f"v{t}", name=f"v{t}") for t in range(NKC)]
        # stacked O^T: feature-chunk jc holds heads (2jc, 2jc+1) on
        # partitions 0-63 / 64-127
        OTS = [qkvp.tile([128, T], BF16, tag=f"ots{j}", name=f"ots{j}")
               for j in range(2)]
        xt_t = [xtp.tile([128, T], BF16, tag=f"x{k}", name=f"x{k}")
                for k in range(NDC)]

        # DMA order matches consumption: Q needs wq + the first x column
        # halves (tbp=0), then the second halves, wk, wv, masks (attention
        # start), wp (projection).
        for k in range(NDC):
            nc.sync.dma_start(wq_t[k][:], wq[k * 128:(k + 1) * 128, :])
            nc.sync.dma_start(xt_t[k][:, 0:1024],
                              xT[k * 128:(k + 1) * 128, 0:1024])
        for k in range(NDC):
            nc.sync.dma_start(xt_t[k][:, 1024:2048],
                              xT[k * 128:(k + 1) * 128, 1024:2048])
        for k in range(NDC):
            nc.sync.dma_start(wk_t[k][:], wk[k * 128:(k + 1) * 128, :])
        for k in range(NDC):
            nc.sync.dma_start(wv_t[k][:], wv[k * 128:(k + 1) * 128, :])
        for i in range(NQB):
            nc.sync.dma_start(mask_t[i][:], msk[i])
        for j in range(2):
            nc.sync.dma_start(wp_t[j][:], wp[j])

        # ---------------- QKV projection pieces ----------------
        nqk = [0]

        def qk_group(W, OUT, fc, tbp):
            # full [128,1024] group in the scores PSUM pool (prefix only)
            ps = psS.tile([128, 1024], F32, tag="s", name="s")
            for j in (0, 1):
                tb = 2 * tbp + j
                for k in range(NDC):
                    nc.tensor.matmul(
                        ps[:, j * 512:(j + 1) * 512],
                        W[k][:, fc * 128:(fc + 1) * 128],
                        xt_t[k][:, tb * 512:(tb + 1) * 512],
                        start=(k == 0), stop=(k == NDC - 1))
            dst = OUT[fc][:, tbp * 1024:(tbp + 1) * 1024]
            if nqk[0] % 2 == 0:
                nc.scalar.copy(dst, ps[:])
            else:
                nc.vector.tensor_copy(dst, ps[:])
            nqk[0] += 1

        def v_tile(tt):
            def emit():
                ps = psX.tile([128, 260], F32, tag="b", name="b")
                for k in range(NDC):
                    nc.tensor.matmul(
                        ps[:], xt_t[k][:, tt * 128:(tt + 1) * 128], wv_t[k][:],
                        start=(k == 0), stop=(k == NDC - 1))
                if tt < 4 and tt % 2 == 0:
                    # prefix only: scalar is free before attention starts
                    nc.scalar.copy(V[tt][:], ps[:])
                else:
                    nc.vector.tensor_copy(V[tt][:], ps[:])
                nc.vector.tensor_copy(
                    V[tt].rearrange("p (h c) -> p h c", c=65)[:, :, 64:65],
                    onesv[:].rearrange("p (h c) -> p h c", c=1))
            return emit

        def pe_filler(n):
            # dependency-free matmuls that keep the HAM activity window
            # busy across short DMA/semaphore stalls (idle windows throttle
            # the PE clock to 1.2 GHz for ~10us)
            psd = psO.tile([65, 512], F32, tag="o0", name="o0")
            for i in range(n):
                nc.tensor.matmul(psd[:], wmt[:, 0:65], wmt[:],
                                 start=(i == 0), stop=(i == n - 1))

        # phase-B prefix: the full Q/K projections (heavy 16-MM groups do
        # not interleave well with the exp-paced attention stream) plus the
        # first four V tiles; V tiles 4-15 are light enough to inject.
        ng = 0
        for W, OUT in ((wq_t, QT), (wk_t, KT)):
            for tbp in range(2):
                for fc in range(2):
                    qk_group(W, OUT, fc, tbp)
                    ng += 1
                    if 1 <= ng <= 5:
                        pe_filler(3)
        for tt in range(4):
            v_tile(tt)()

        # ---------------- attention ----------------
        ptp = ctx.enter_context(tc.tile_pool(name="pt", bufs=5))
        rcp = ctx.enter_context(tc.tile_pool(name="rcp", bufs=2))

        def make_norm_steps(hp, dds, ou_all):
            def norm_step(h, qb, fast=False):
                def emit():
                    hh = h % 2
                    dd = dds[(h, qb)]
                    psb = psX.tile([64, 512], F32, tag="b", name="b")
                    nc.tensor.matmul(
                        psb[:], ones_t[0:1, 0:64], dd[0:1, :],
                        start=True, stop=True)
                    if hh == 0:
                        nc.vector.tensor_tensor(
                            OTS[hp][0:64, qb * 512:(qb + 1) * 512],
                            ou_all[(h, qb)][0:64, :], psb[:], AL.mult)
                    else:
                        # odd head: normalize to a bounce tile, DMA to
                        # partitions 64-127 of the stacked O^T; in the
                        # drain tail use the scalar engine's DGE queue so
                        # it never waits behind bulk y-tile DMAs
                        ob = rcp.tile([64, 512], BF16, tag="ob",
                                      name="ob", bufs=2)
                        nc.vector.tensor_tensor(
                            ob[:], ou_all[(h, qb)][0:64, :], psb[:],
                            AL.mult)
                        nc.sync.dma_start(
                            OTS[hp][64:128, qb * 512:(qb + 1) * 512],
                            ob[:])
                return emit
            return norm_step

        def proj_tile(tt, mb, ybp, on_scalar=False):
            psy = psX.tile([128, 512], F32, tag="b", name="yps")
            for jc in range(2):
                nc.tensor.matmul(
                    psy[:],
                    OTS[jc][:, tt * 128:(tt + 1) * 128],
                    wp_t[jc][:, mb * 512:(mb + 1) * 512],
                    start=(jc == 0), stop=(jc == 1))
            yt = ybp.tile([128, 512], BF16, tag="yt", name="yt")
            if on_scalar:
                nc.scalar.copy(yt[:], psy[:])
            else:
                nc.vector.tensor_copy(yt[:], psy[:])
            nc.sync.dma_start(
                y[tt * 128:(tt + 1) * 128, mb * 512:(mb + 1) * 512],
                yt[:])

        ybp = ctx.enter_context(tc.tile_pool(name="yb", bufs=3))
        # leftover V-projection tiles, interleaved into hp0's attention
        inject = [v_tile(t) for t in range(4, NKC)]

        for hp in range(HPC // 2):
            fc = hp
            heads = (2 * hp, 2 * hp + 1)
            dds = {}
            ou_all = {}
            norm_step = make_norm_steps(hp, dds, ou_all)
            for qb in range(NQB):
                if hp == 0 and qb == NQB - 1:
                    # hp0's qb0-2 normalize work runs during its own qb3
                    inject += [norm_step(h, q)
                               for q in range(3) for h in heads]
                nkc = 4 * (qb + 1)
                # hp0 paces the injected V tiles just-in-time (4 per query
                # block) so they fill PE slack instead of starving exp;
                # hp1 and hp0-qb3 drain greedily
                npop = 4 if (hp == 0 and qb in (1, 2)) else \
                    0 if (hp == 0 and qb == 0) else 999
                pso = {h: psO.tile([65, 512], F32, tag=f"o{h % 2}",
                                   name=f"o{h % 2}") for h in heads}
                ptq = {}
                for kc in range(nkc + LAG):
                    if kc < nkc:
                        # both heads' S^T for this key chunk in one PSUM
                        # tile; the two K=64 matmuls alternate PE row
                        # groups and run concurrently
                        pss = psS.tile([128, 1024], F32, tag="s", name="s")
                        for h in heads:
                            po = 64 * (h % 2)
                            nc.tensor.matmul(
                                pss[:, po * 8:po * 8 + 512],
                                KT[fc][po:po + 64, kc * 128:(kc + 1) * 128],
                                QT[fc][po:po + 64, qb * 512:(qb + 1) * 512],
                                start=True, stop=True)
                        pt = ptp.tile([128, 1024], BF16, tag="pt", name="pt")
                        nc.scalar.activation(pt[:], pss[:], EXP)
                        if kc >= 4 * qb:  # diagonal chunk -> causal mask
                            # only cols < 128*(t+1) can be zeroed (beyond
                            # that the causal mask is all-ones), so slice
                            # the multiply to the affected prefix per head
                            t = kc - 4 * qb
                            w = 128 * (t + 1)
                            if w >= 512:
                                nc.vector.tensor_tensor(
                                    pt[:], pt[:], mask_t[t][:], AL.mult)
                            else:
                                pts = pt.rearrange(
                                    "p (g c) -> p g c", c=512)[:, :, 0:w]
                                nc.vector.tensor_tensor(
                                    pts, pts,
                                    mask_t[t].rearrange(
                                        "p (g c) -> p g c", c=512)[:, :, 0:w],
                                    AL.mult)
                        ptq[kc] = pt
                    kcp = kc - LAG
                    if kcp >= 0 and kcp in ptq:
                        ptv = ptq.pop(kcp)
                        for h in heads:
                            po = 64 * (h % 2)
                            nc.tensor.matmul(
                                pso[h][:],
                                V[kcp][:, 65 * h:65 * h + 65],
                                ptv[:, po * 8:po * 8 + 512],
                                start=(kcp == 0),
                                stop=(kcp == nkc - 1))
                    if kcp >= 0 and inject and npop > 0:
                        npop -= 1
                        inject.pop(0)()
                        if len(inject) > 10 and (hp == 1 or qb == NQB - 1):
                            inject.pop(0)()
                for h in heads:
                    # O rows to SBUF; denominator row DMA'd from PSUM to a
                    # partition-0 tile (DVE ops are lane-wise and cannot
                    # shift partitions; the DMA can)
                    ou = rcp.tile([65, 512], F32,
                                  tag=f"ou{hp}_{h % 2}_{qb}",
                                  name=f"ou{hp}_{h % 2}_{qb}", bufs=1)
                    # scalar: at block transitions the exp stream idles
                    # (next block's scores aren't done), so the copy is
                    # ~free there and releases the PV accumulator ~1.5us
                    # earlier than the congested vector queue would
                    nc.scalar.copy(ou[:], pso[h][:])
                    den = rcp.tile([1, 512], F32, tag="den",
                                   name="den", bufs=2)
                    nc.sync.dma_start(den[:], ou[64:65, :])
                    dhr = rcp.tile([1, 512], F32, tag="dhr",
                                   name="dhr", bufs=2)
                    nc.vector.reciprocal_approx_fast(dhr[:], den[:])
                    dd = rcp.tile([1, 512], BF16,
                                  tag=f"dd{hp}_{h % 2}_{qb}",
                                  name=f"dd{hp}_{h % 2}_{qb}", bufs=1)
                    nc.vector.tensor_copy(dd[:], dhr[:])
                    dds[(h, qb)] = dd
                    ou_all[(h, qb)] = ou
                if hp == 0 and qb == NQB - 1:
                    inject += [norm_step(h, 3) for h in heads]
                if hp == 1:
                    # each query block's normalize + projection batch is
                    # ready as soon as that block's attention finishes;
                    # emitting per-qb keeps every block's slots evenly fed
                    last = qb == NQB - 1
                    inject += [norm_step(h, qb, fast=last) for h in heads]
                    inject += [(lambda t=t, m=m, s=last:
                                proj_tile(t, m, ybp,
                                          on_scalar=s and (t + m) % 2 == 0))
                               for t in range(4 * qb, 4 * qb + 4)
                               for m in range(2)]
        # drain any remaining injected steps (tail of the kernel); filler
        # matmuls bridge the denominator/normalize dependency chain so the
        # PE clock stays at 2.4 GHz for the final projection tiles
        pe_filler(14)
        ndrain = 0
        while inject:
            inject.pop(0)()
            ndrain += 1
            if ndrain == 2:
                pe_filler(6)
        ctx.close()

    nc.finalize()
    return nc


def make_masks():
    """[NQB, 128, 1024]: mask for diagonal chunk offset t, duplicated for the
    two heads (cols 0-511 and 512-1023 identical).
    keep iff query_in_block >= key_in_chunk + 128*t."""
    m = np.zeros((NQB, 128, 1024), dtype=np.float32)
    f = np.arange(512)
    p = np.arange(128)
    for t in range(NQB):
        pat = (f[None, :] >= p[:, None] + 128 * t).astype(np.float32)
        m[t][:, 0:512] = pat
        m[t][:, 512:1024] = pat
    return m


def shard_inputs(x, Wqkv, bqkv, Wproj):
    x = np.asarray(x, dtype=np.float32)
    Wqkv = np.asarray(Wqkv, dtype=np.float32)
    bqkv = np.asarray(bqkv, dtype=np.float32)
    Wproj = np.asarray(Wproj, dtype=np.float32)
    assert not np.any(bqkv[0:2048]), \
        "nonzero q/k bias not supported by the fast kernel"
    bf = ml_dtypes.bfloat16
    masks = make_masks().astype(bf)
    in_maps = []
    for c in range(8):
        b, g = c // 4, c % 4
        cs = slice(256 * g, 256 * g + 256)
        wq_ = np.ascontiguousarray(Wqkv[:, 0:1024][:, cs]) / 8.0
        wk_ = np.ascontiguousarray(Wqkv[:, 1024:2048][:, cs])
        wv_src = Wqkv[:, 2048:3072][:, cs]
        wv_ = np.zeros((DM, 260), dtype=np.float32)
        for h in range(4):
            wv_[:, 65 * h:65 * h + 64] = wv_src[:, 64 * h:64 * h + 64]
        wp_ = np.ascontiguousarray(
            Wproj[256 * g:256 * g + 256, :].reshape(2, 128, DM))
        in_maps.append({
            "xT": np.ascontiguousarray(x[b].T).astype(bf),
            "wq": wq_.astype(bf), "wk": wk_.astype(bf),
            "wv": wv_.astype(bf), "wp": wp_.astype(bf), "msk": masks,
        })
    return in_maps


def combine_outputs(results, Wqkv, bqkv, Wproj, bproj):
    bqkv = np.asarray(bqkv, dtype=np.float32)
    Wproj = np.asarray(Wproj, dtype=np.float32)
    bproj = np.asarray(bproj, dtype=np.float32)
    bv_term = bqkv[2048:3072] @ Wproj
    out = np.zeros((2, T, DM), dtype=np.float32)
    for c in range(8):
        out[c // 4] += results[c]["y"].astype(np.float32)
    out += (bv_term + bproj)[None, None, :]
    return out


_NC_CACHE = []


def _numpy_fallback(x, Wqkv, bqkv, Wproj, bproj):
    # exact-but-slow path for inputs the device kernel does not support
    b, t, dm = x.shape
    h, d = 16, 64
    qkv = x @ Wqkv + bqkv
    q, k, v = np.split(qkv, 3, axis=-1)
    q = q.reshape(b, t, h, d).transpose(0, 2, 1, 3)
    k = k.reshape(b, t, h, d).transpose(0, 2, 1, 3)
    v = v.reshape(b, t, h, d).transpose(0, 2, 1, 3)
    att = np.einsum('bhqd,bhkd->bhqk', q, k) / np.sqrt(np.float32(d))
    causal = np.tril(np.ones((t, t), dtype=bool))
    att = np.where(causal[None, None], att, -np.inf)
    att = att - att.max(axis=-1, keepdims=True)
    e = np.exp(att)
    p = e / e.sum(axis=-1, keepdims=True)
    out = np.einsum('bhqk,bhkd->bhqd', p, v)
    out = out.transpose(0, 2, 1, 3).reshape(b, t, dm)
    return (out @ Wproj + bproj).astype(np.float32)


def kernel(x, Wqkv, bqkv, Wproj, bproj):
    x = np.asarray(x, dtype=np.float32)
    Wqkv = np.asarray(Wqkv, dtype=np.float32)
    bqkv = np.asarray(bqkv, dtype=np.float32)
    Wproj = np.asarray(Wproj, dtype=np.float32)
    bproj = np.asarray(bproj, dtype=np.float32)
    if np.any(bqkv[0:2048]):
        # nonzero q/k bias falls outside the fused device kernel's contract
        return _numpy_fallback(x, Wqkv, bqkv, Wproj, bproj)
    from concourse.bass_utils import run_bass_kernel_spmd
    if not _NC_CACHE:
        _NC_CACHE.append(build_nc())
    nc = _NC_CACHE[0]
    in_maps = shard_inputs(x, Wqkv, bqkv, Wproj)
    res = run_bass_kernel_spmd(nc, in_maps, core_ids=list(range(8)))
    return combine_outputs(res.results, Wqkv, bqkv, Wproj, bproj)


# revision 47
# speedup vs baseline: 1.1771x; 1.1771x over previous
"""Trainium2 Bass kernel: multi-head self-attention (B=2, T=2048, D=1024, H=16),
8-core SPMD. Accepts FULL inputs, returns the FULL output.

Sharding: data-parallel over batch (2) x tensor-parallel over heads (4 groups
of 4). Each core computes attention for its 4 heads of one batch plus its
partial output projection; the host sums the 4 partials per batch (plus the
bias terms, folded exactly). Matmuls run in bf16 on the PE (2x the fp32r
stream rate); accumulation is fp32 in PSUM and softmax denominators stay fp32.

Schedule: a short phase-B prefix (Q/K for head-pair 0 over the first T half,
V tiles 0-3) runs as soon as its DMAs land; everything else (remaining QKV
projection tiles, normalization, output projection) is interleaved into the
attention key-chunk stream, which is paced by the exp() activations on the
scalar engine. Causal-mask multiplies run on the otherwise-idle GpSimd engine
so the P-tile critical path never queues behind vector-engine copies.
"""
import sys
if '/opt/trn_rl_repo' not in sys.path:
    sys.path.insert(0, '/opt/trn_rl_repo')
import numpy as np
import ml_dtypes
import concourse.bass as bass
import concourse.mybir as mybir
from concourse import bacc
from concourse.tile import TileContext

F32 = mybir.dt.float32
F32R = mybir.dt.float32r
BF16 = mybir.dt.bfloat16
AL = mybir.AluOpType
EXP = mybir.ActivationFunctionType.Exp

T = 2048
DM = 1024
HPC = 4
D = 64
NQB = 4           # query blocks of 512
NKC = 16          # key chunks of 128
NDC = 8           # contraction chunks of 128 for projections
LAG = 3           # PV lags S/exp by this many key chunks


def build_nc():
    nc = bacc.Bacc("TRN2", target_bir_lowering=False, debug=True)

    xT = nc.dram_tensor("xT", [DM, T], BF16, kind="ExternalInput")
    wq = nc.dram_tensor("wq", [DM, 256], BF16, kind="ExternalInput")
    wk = nc.dram_tensor("wk", [DM, 256], BF16, kind="ExternalInput")
    wv = nc.dram_tensor("wv", [DM, 260], BF16, kind="ExternalInput")
    wp = nc.dram_tensor("wp", [2, 128, DM], BF16, kind="ExternalInput")
    msk = nc.dram_tensor("msk", [NQB, 128, 1024], BF16, kind="ExternalInput")
    y = nc.dram_tensor("y", [T, DM], BF16, kind="ExternalOutput")

    with nc.allow_low_precision("bf16 matmul pipeline"), TileContext(nc) as tc:
        from contextlib import ExitStack
        ctx = ExitStack()
        cp = ctx.enter_context(tc.tile_pool(name="const", bufs=1))
        wtp = ctx.enter_context(tc.tile_pool(name="wts", bufs=1))
        qkvp = ctx.enter_context(tc.tile_pool(name="qkv", bufs=1))
        xtp = ctx.enter_context(tc.tile_pool(name="xt", bufs=1))
        psS = ctx.enter_context(tc.tile_pool(name="psS", bufs=2, space="PSUM"))
        psO = ctx.enter_context(tc.tile_pool(name="psO", bufs=1, space="PSUM"))
        psX = ctx.enter_context(tc.tile_pool(name="psX", bufs=2, space="PSUM"))

        mask_t = [cp.tile([128, 1024], BF16, tag=f"m{i}", name=f"m{i}")
                  for i in range(NQB)]
        ones32 = cp.tile([128, 64], F32, tag="ones32", name="ones32")
        nc.vector.memset(ones32[:], 1.0)
        # preload the exp ACT table set while the input DMAs stream in
        warm = cp.tile([1, 8], F32, tag="warm", name="warm")
        nc.scalar.activation(warm[:], ones32[0:1, 0:8], EXP)
        ones_t = cp.tile([128, 64], BF16, tag="ones", name="ones")
        nc.vector.tensor_copy(ones_t[:], ones32[:])
        onesv = cp.tile([128, 4], F32, tag="onesv", name="onesv")
        nc.vector.memset(onesv[:], 1.0)
        # PE warm-up: dummy matmuls with no DMA deps flip the HAM clock
        # gate to 8/8 (2.4 GHz) before the real streams begin
        wmt = cp.tile([128, 512], BF16, tag="wmt", name="wmt")
        nc.vector.memset(wmt[:], 0.0)
        psW = psX.tile([128, 512], F32, tag="b", name="b")
        for i in range(20):
            # ~6us: bridges the input-DMA ramp so the PE never idles a
            # full MID window before the first projection group
            nc.tensor.matmul(psW[:], wmt[:, 0:128], wmt[:],
                             start=(i == 0), stop=(i == 19))

        # weights
        wq_t = [wtp.tile([128, 256], BF16, tag=f"wq{k}", name=f"wq{k}")
                for k in range(NDC)]
        wk_t = [wtp.tile([128, 256], BF16, tag=f"wk{k}", name=f"wk{k}")
                for k in range(NDC)]
        wv_t = [wtp.tile([128, 260], BF16, tag=f"wv{k}", name=f"wv{k}")
                for k in range(NDC)]
        wp_t = [wtp.tile([128, DM], BF16, tag=f"wp{j}", name=f"wp{j}")
                for j in range(2)]

        # persistent activations
        QT = [qkvp.tile([128, T], BF16, tag=f"qt{i}", name=f"qt{i}") for i in range(2)]
        KT = [qkvp.tile([128, T], BF16, tag=f"kt{i}", name=f"kt{i}") for i in range(2)]
        V = [qkvp.tile([128, 260], BF16, tag=f"v{t}", name=f"v{t}") for t in range(NKC)]
        # stacked O^T: feature-chunk jc holds heads (2jc, 2jc+1) on
        # partitions 0-63 / 64-127
        OTS = [qkvp.tile([128, T], BF16, tag=f"ots{j}", name=f"ots{j}")
               for j in range(2)]
        xt_t = [xtp.tile([128, T], BF16, tag=f"x{k}", name=f"x{k}")
                for k in range(NDC)]

        # DMA order matches consumption: Q needs wq + the first x column
        # halves (tbp=0), then the second halves, wk, wv, masks (attention
        # start), wp (projection).
        for k in range(NDC):
            nc.sync.dma_start(wq_t[k][:], wq[k * 128:(k + 1) * 128, :])
            nc.sync.dma_start(xt_t[k][:, 0:1024],
                              xT[k * 128:(k + 1) * 128, 0:1024])
        for k in range(NDC):
            nc.sync.dma_start(xt_t[k][:, 1024:2048],
                              xT[k * 128:(k + 1) * 128, 1024:2048])
        for k in range(NDC):
            nc.sync.dma_start(wk_t[k][:], wk[k * 128:(k + 1) * 128, :])
        for k in range(NDC):
            nc.sync.dma_start(wv_t[k][:], wv[k * 128:(k + 1) * 128, :])
        for i in range(NQB):
            nc.sync.dma_start(mask_t[i][:], msk[i])
        for j in range(2):
            nc.sync.dma_start(wp_t[j][:], wp[j])

        # ---------------- QKV projection pieces ----------------
        nqk = [0]

        def qk_group(W, OUT, fc, tbp):
            # full [128,1024] group in the scores PSUM pool (prefix only)
            ps = psS.tile([128, 1024], F32, tag="s", name="s")
            for j in (0, 1):
                tb = 2 * tbp + j
                for k in range(NDC):
                    nc.tensor.matmul(
                        ps[:, j * 512:(j + 1) * 512],
                        W[k][:, fc * 128:(fc + 1) * 128],
                        xt_t[k][:, tb * 512:(tb + 1) * 512],
                        start=(k == 0), stop=(k == NDC - 1))
            dst = OUT[fc][:, tbp * 1024:(tbp + 1) * 1024]
            if nqk[0] % 2 == 0:
                nc.scalar.copy(dst, ps[:])
            else:
                nc.vector.tensor_copy(dst, ps[:])
            nqk[0] += 1

        def v_tile(tt):
            def emit():
                ps = psX.tile([128, 260], F32, tag="b", name="b")
                for k in range(NDC):
                    nc.tensor.matmul(
                        ps[:], xt_t[k][:, tt * 128:(tt + 1) * 128], wv_t[k][:],
                        start=(k == 0), stop=(k == NDC - 1))
                if tt < 4 and tt % 2 == 0:
                    # prefix only: scalar is free before attention starts
                    nc.scalar.copy(V[tt][:], ps[:])
                else:
                    nc.vector.tensor_copy(V[tt][:], ps[:])
                nc.vector.tensor_copy(
                    V[tt].rearrange("p (h c) -> p h c", c=65)[:, :, 64:65],
                    onesv[:].rearrange("p (h c) -> p h c", c=1))
            return emit

        def pe_filler(n):
            # dependency-free matmuls that keep the HAM activity window
            # busy across short DMA/semaphore stalls (idle windows throttle
            # the PE clock to 1.2 GHz for ~10us)
            psd = psO.tile([65, 512], F32, tag="o0", name="o0")
            for i in range(n):
                nc.tensor.matmul(psd[:], wmt[:, 0:65], wmt[:],
                                 start=(i == 0), stop=(i == n - 1))

        def qk_half(W, OUT, fc, tbp, j):
            # half-width group in the rotating psX pool: safe to interleave
            # with the attention stream (never holds PSUM across slots)
            def emit():
                tb = 2 * tbp + j
                ps = psX.tile([128, 512], F32, tag="b", name="b")
                for k in range(NDC):
                    nc.tensor.matmul(
                        ps[:],
                        W[k][:, fc * 128:(fc + 1) * 128],
                        xt_t[k][:, tb * 512:(tb + 1) * 512],
                        start=(k == 0), stop=(k == NDC - 1))
                c0 = tbp * 1024 + j * 512
                nc.vector.tensor_copy(OUT[fc][:, c0:c0 + 512], ps[:])
            return emit

        # phase-B prefix: Q/K for head-pair 0 only; head-pair 1's Q/K and
        # V tiles 4-15 are injected into hp0's attention slack just-in-time
        ng = 0
        for W, OUT in ((wq_t, QT), (wk_t, KT)):
            for tbp in range(2):
                qk_group(W, OUT, 0, tbp)
                ng += 1
                if 1 <= ng <= 4:
                    pe_filler(3)
        for tt in range(4):
            v_tile(tt)()

        # ---------------- attention ----------------
        ptp = ctx.enter_context(tc.tile_pool(name="pt", bufs=5))
        rcp = ctx.enter_context(tc.tile_pool(name="rcp", bufs=2))

        def make_norm_steps(hp, dds, ou_all):
            def norm_step(h, qb, fast=False):
                def emit():
                    hh = h % 2
                    dd = dds[(h, qb)]
                    psb = psX.tile([64, 512], F32, tag="b", name="b")
                    nc.tensor.matmul(
                        psb[:], ones_t[0:1, 0:64], dd[0:1, :],
                        start=True, stop=True)
                    if hh == 0:
                        nc.vector.tensor_tensor(
                            OTS[hp][0:64, qb * 512:(qb + 1) * 512],
                            ou_all[(h, qb)][0:64, :], psb[:], AL.mult)
                    else:
                        # odd head: normalize to a bounce tile, DMA to
                        # partitions 64-127 of the stacked O^T; in the
                        # drain tail use the scalar engine's DGE queue so
                        # it never waits behind bulk y-tile DMAs
                        ob = rcp.tile([64, 512], BF16, tag="ob",
                                      name="ob", bufs=2)
                        nc.vector.tensor_tensor(
                            ob[:], ou_all[(h, qb)][0:64, :], psb[:],
                            AL.mult)
                        nc.sync.dma_start(
                            OTS[hp][64:128, qb * 512:(qb + 1) * 512],
                            ob[:])
                return emit
            return norm_step

        def proj_tile(tt, mb, ybp, on_scalar=False):
            psy = psX.tile([128, 512], F32, tag="b", name="yps")
            for jc in range(2):
                nc.tensor.matmul(
                    psy[:],
                    OTS[jc][:, tt * 128:(tt + 1) * 128],
                    wp_t[jc][:, mb * 512:(mb + 1) * 512],
                    start=(jc == 0), stop=(jc == 1))
            yt = ybp.tile([128, 512], BF16, tag="yt", name="yt")
            if on_scalar:
                nc.scalar.copy(yt[:], psy[:])
            else:
                nc.vector.tensor_copy(yt[:], psy[:])
            nc.sync.dma_start(
                y[tt * 128:(tt + 1) * 128, mb * 512:(mb + 1) * 512],
                yt[:])

        ybp = ctx.enter_context(tc.tile_pool(name="yb", bufs=3))
        # leftover V tiles + head-pair-1 Q/K halves, ordered by need-by
        # slot (V4-7 during qb1, Kfc1 complete before hp1 starts)
        qh = [qk_half(wq_t, QT, 1, tbp, j) for tbp in (0, 1) for j in (0, 1)]
        kh = [qk_half(wk_t, KT, 1, tbp, j) for tbp in (0, 1) for j in (0, 1)]
        vt = [v_tile(t) for t in range(4, NKC)]
        inject = [vt[0], vt[1], qh[0], vt[2], vt[3], qh[1],
                  vt[4], qh[2], vt[5], qh[3], vt[6], kh[0], vt[7], kh[1],
                  kh[2], kh[3], vt[8], vt[9], vt[10], vt[11]]

        for hp in range(HPC // 2):
            fc = hp
            heads = (2 * hp, 2 * hp + 1)
            dds = {}
            ou_all = {}
            norm_step = make_norm_steps(hp, dds, ou_all)
            for qb in range(NQB):
                if hp == 0 and qb == NQB - 1:
                    # hp0's qb0-2 normalize work runs during its own qb3
                    inject += [norm_step(h, q)
                               for q in range(3) for h in heads]
                nkc = 4 * (qb + 1)
                # hp0 paces the injected projection pieces just-in-time so
                # they fill PE slack instead of starving exp; hp1 and
                # hp0-qb3 drain greedily
                npop = {0: 2, 1: 4, 2: 8, 3: 999}[qb] if hp == 0 else 999
                pso = {h: psO.tile([65, 512], F32, tag=f"o{h % 2}",
                                   name=f"o{h % 2}") for h in heads}
                ptq = {}
                for kc in range(nkc + LAG):
                    if kc < nkc:
                        # both heads' S^T for this key chunk in one PSUM
                        # tile; the two K=64 matmuls alternate PE row
                        # groups and run concurrently
                        pss = psS.tile([128, 1024], F32, tag="s", name="s")
                        for h in heads:
                            po = 64 * (h % 2)
                            nc.tensor.matmul(
                                pss[:, po * 8:po * 8 + 512],
                                KT[fc][po:po + 64, kc * 128:(kc + 1) * 128],
                                QT[fc][po:po + 64, qb * 512:(qb + 1) * 512],
                                start=True, stop=True)
                        pt = ptp.tile([128, 1024], BF16, tag="pt", name="pt")
                        nc.scalar.activation(pt[:], pss[:], EXP)
                        if kc >= 4 * qb:  # diagonal chunk -> causal mask
                            # only cols < 128*(t+1) can be zeroed (beyond
                            # that the causal mask is all-ones), so slice
                            # the multiply to the affected prefix per head
                            t = kc - 4 * qb
                            w = 128 * (t + 1)
                            if w >= 512:
                                nc.vector.tensor_tensor(
                                    pt[:], pt[:], mask_t[t][:], AL.mult)
                            else:
                                pts = pt.rearrange(
                                    "p (g c) -> p g c", c=512)[:, :, 0:w]
                                nc.vector.tensor_tensor(
                                    pts, pts,
                                    mask_t[t].rearrange(
                                        "p (g c) -> p g c", c=512)[:, :, 0:w],
                                    AL.mult)
                        ptq[kc] = pt
                    kcp = kc - LAG
                    if kcp >= 0 and kcp in ptq:
                        ptv = ptq.pop(kcp)
                        for h in heads:
                            po = 64 * (h % 2)
                            nc.tensor.matmul(
                                pso[h][:],
                                V[kcp][:, 65 * h:65 * h + 65],
                                ptv[:, po * 8:po * 8 + 512],
                                start=(kcp == 0),
                                stop=(kcp == nkc - 1))
                    if kcp >= 0 and inject and npop > 0:
                        npop -= 1
                        inject.pop(0)()
                        if len(inject) > 10 and (hp == 1 or qb == NQB - 1):
                            inject.pop(0)()
                for h in heads:
                    # O rows to SBUF; denominator row DMA'd from PSUM to a
                    # partition-0 tile (DVE ops are lane-wise and cannot
                    # shift partitions; the DMA can)
                    ou = rcp.tile([65, 512], F32,
                                  tag=f"ou{hp}_{h % 2}_{qb}",
                                  name=f"ou{hp}_{h % 2}_{qb}", bufs=1)
                    if hp == 1 and qb == 3:
                        # tail: scalar is idle (exp done) and the vector
                        # queue is full of projection copies
                        nc.scalar.copy(ou[:], pso[h][:])
                    else:
                        nc.vector.tensor_copy(ou[:], pso[h][:])
                    den = rcp.tile([1, 512], F32, tag="den",
                                   name="den", bufs=2)
                    nc.sync.dma_start(den[:], ou[64:65, :])
                    dhr = rcp.tile([1, 512], F32, tag="dhr",
                                   name="dhr", bufs=2)
                    nc.vector.reciprocal_approx_fast(dhr[:], den[:])
                    dd = rcp.tile([1, 512], BF16,
                                  tag=f"dd{hp}_{h % 2}_{qb}",
                                  name=f"dd{hp}_{h % 2}_{qb}", bufs=1)
                    nc.vector.tensor_copy(dd[:], dhr[:])
                    dds[(h, qb)] = dd
                    ou_all[(h, qb)] = ou
                if hp == 0 and qb == NQB - 1:
                    inject += [norm_step(h, 3) for h in heads]
                if hp == 1:
                    # each query block's normalize + projection batch is
                    # ready as soon as that block's attention finishes;
                    # emitting per-qb keeps every block's slots evenly fed
                    last = qb == NQB - 1
                    inject += [norm_step(h, qb, fast=last) for h in heads]
                    inject += [(lambda t=t, m=m, s=last:
                                proj_tile(t, m, ybp,
                                          on_scalar=s and (t + m) % 2 == 0))
                               for t in range(4 * qb, 4 * qb + 4)
                               for m in range(2)]
        # drain any remaining injected steps (tail of the kernel); filler
        # matmuls bridge the denominator/normalize dependency chain so the
        # PE clock stays at 2.4 GHz for the final projection tiles
        pe_filler(14)
        ndrain = 0
        while inject:
            inject.pop(0)()
            ndrain += 1
            if ndrain == 2:
                pe_filler(6)
        ctx.close()

    nc.finalize()
    return nc


def make_masks():
    """[NQB, 128, 1024]: mask for diagonal chunk offset t, duplicated for the
    two heads (cols 0-511 and 512-1023 identical).
    keep iff query_in_block >= key_in_chunk + 128*t."""
    m = np.zeros((NQB, 128, 1024), dtype=np.float32)
    f = np.arange(512)
    p = np.arange(128)
    for t in range(NQB):
        pat = (f[None, :] >= p[:, None] + 128 * t).astype(np.float32)
        m[t][:, 0:512] = pat
        m[t][:, 512:1024] = pat
    return m


def shard_inputs(x, Wqkv, bqkv, Wproj):
    x = np.asarray(x, dtype=np.float32)
    Wqkv = np.asarray(Wqkv, dtype=np.float32)
    bqkv = np.asarray(bqkv, dtype=np.float32)
    Wproj = np.asarray(Wproj, dtype=np.float32)
    assert not np.any(bqkv[0:2048]), \
        "nonzero q/k bias not supported by the fast kernel"
    bf = ml_dtypes.bfloat16
    masks = make_masks().astype(bf)
    in_maps = []
    for c in range(8):
        b, g = c // 4, c % 4
        cs = slice(256 * g, 256 * g + 256)
        wq_ = np.ascontiguousarray(Wqkv[:, 0:1024][:, cs]) / 8.0
        wk_ = np.ascontiguousarray(Wqkv[:, 1024:2048][:, cs])
        wv_src = Wqkv[:, 2048:3072][:, cs]
        wv_ = np.zeros((DM, 260), dtype=np.float32)
        for h in range(4):
            wv_[:, 65 * h:65 * h + 64] = wv_src[:, 64 * h:64 * h + 64]
        wp_ = np.ascontiguousarray(
            Wproj[256 * g:256 * g + 256, :].reshape(2, 128, DM))
        in_maps.append({
            "xT": np.ascontiguousarray(x[b].T).astype(bf),
            "wq": wq_.astype(bf), "wk": wk_.astype(bf),
            "wv": wv_.astype(bf), "wp": wp_.astype(bf), "msk": masks,
        })
    return in_maps


def combine_outputs(results, Wqkv, bqkv, Wproj, bproj):
    bqkv = np.asarray(bqkv, dtype=np.float32)
    Wproj = np.asarray(Wproj, dtype=np.float32)
    bproj = np.asarray(bproj, dtype=np.float32)
    bv_term = bqkv[2048:3072] @ Wproj
    out = np.zeros((2, T, DM), dtype=np.float32)
    for c in range(8):
        out[c // 4] += results[c]["y"].astype(np.float32)
    out += (bv_term + bproj)[None, None, :]
    return out


_NC_CACHE = []


def _numpy_fallback(x, Wqkv, bqkv, Wproj, bproj):
    # exact-but-slow path for inputs the device kernel does not support
    b, t, dm = x.shape
    h, d = 16, 64
    qkv = x @ Wqkv + bqkv
    q, k, v = np.split(qkv, 3, axis=-1)
    q = q.reshape(b, t, h, d).transpose(0, 2, 1, 3)
    k = k.reshape(b, t, h, d).transpose(0, 2, 1, 3)
    v = v.reshape(b, t, h, d).transpose(0, 2, 1, 3)
    att = np.einsum('bhqd,bhkd->bhqk', q, k) / np.sqrt(np.float32(d))
    causal = np.tril(np.ones((t, t), dtype=bool))
    att = np.where(causal[None, None], att, -np.inf)
    att = att - att.max(axis=-1, keepdims=True)
    e = np.exp(att)
    p = e / e.sum(axis=-1, keepdims=True)
    out = np.einsum('bhqk,bhkd->bhqd', p, v)
    out = out.transpose(0, 2, 1, 3).reshape(b, t, dm)
    return (out @ Wproj + bproj).astype(np.float32)


def kernel(x, Wqkv, bqkv, Wproj, bproj):
    x = np.asarray(x, dtype=np.float32)
    Wqkv = np.asarray(Wqkv, dtype=np.float32)
    bqkv = np.asarray(bqkv, dtype=np.float32)
    Wproj = np.asarray(Wproj, dtype=np.float32)
    bproj = np.asarray(bproj, dtype=np.float32)
    if np.any(bqkv[0:2048]):
        # nonzero q/k bias falls outside the fused device kernel's contract
        return _numpy_fallback(x, Wqkv, bqkv, Wproj, bproj)
    from concourse.bass_utils import run_bass_kernel_spmd
    if not _NC_CACHE:
        _NC_CACHE.append(build_nc())
    nc = _NC_CACHE[0]
    in_maps = shard_inputs(x, Wqkv, bqkv, Wproj)
    res = run_bass_kernel_spmd(nc, in_maps, core_ids=list(range(8)))
    return combine_outputs(res.results, Wqkv, bqkv, Wproj, bproj)


# revision 50
# speedup vs baseline: 1.1934x; 1.0138x over previous
"""Trainium2 Bass kernel: multi-head self-attention (B=2, T=2048, D=1024, H=16),
8-core SPMD. Accepts FULL inputs, returns the FULL output.

Sharding: data-parallel over batch (2) x tensor-parallel over heads (4 groups
of 4). Each core computes attention for its 4 heads of one batch plus its
partial output projection; the host sums the 4 partials per batch (plus the
bias terms, folded exactly). Matmuls run in bf16 on the PE (2x the fp32r
stream rate); accumulation is fp32 in PSUM and softmax denominators stay fp32.

Schedule: a short phase-B prefix (Q/K for head-pair 0 over the first T half,
V tiles 0-3) runs as soon as its DMAs land; everything else (remaining QKV
projection tiles, normalization, output projection) is interleaved into the
attention key-chunk stream, which is paced by the exp() activations on the
scalar engine. Causal-mask multiplies run on the otherwise-idle GpSimd engine
so the P-tile critical path never queues behind vector-engine copies.
"""
import sys
if '/opt/trn_rl_repo' not in sys.path:
    sys.path.insert(0, '/opt/trn_rl_repo')
import numpy as np
import ml_dtypes
import concourse.bass as bass
import concourse.mybir as mybir
from concourse import bacc
from concourse.tile import TileContext

F32 = mybir.dt.float32
F32R = mybir.dt.float32r
BF16 = mybir.dt.bfloat16
AL = mybir.AluOpType
EXP = mybir.ActivationFunctionType.Exp

T = 2048
DM = 1024
HPC = 4
D = 64
NQB = 4           # query blocks of 512
NKC = 16          # key chunks of 128
NDC = 8           # contraction chunks of 128 for projections
LAG = 3           # PV lags S/exp by this many key chunks


def build_nc():
    nc = bacc.Bacc("TRN2", target_bir_lowering=False, debug=True)

    xT = nc.dram_tensor("xT", [DM, T], BF16, kind="ExternalInput")
    wq = nc.dram_tensor("wq", [DM, 256], BF16, kind="ExternalInput")
    wk = nc.dram_tensor("wk", [DM, 256], BF16, kind="ExternalInput")
    wv = nc.dram_tensor("wv", [DM, 260], BF16, kind="ExternalInput")
    wp = nc.dram_tensor("wp", [2, 128, DM], BF16, kind="ExternalInput")
    msk = nc.dram_tensor("msk", [NQB, 128, 1024], BF16, kind="ExternalInput")
    y = nc.dram_tensor("y", [T, DM], BF16, kind="ExternalOutput")

    with nc.allow_low_precision("bf16 matmul pipeline"), TileContext(nc) as tc:
        from contextlib import ExitStack
        ctx = ExitStack()
        cp = ctx.enter_context(tc.tile_pool(name="const", bufs=1))
        wtp = ctx.enter_context(tc.tile_pool(name="wts", bufs=1))
        qkvp = ctx.enter_context(tc.tile_pool(name="qkv", bufs=1))
        xtp = ctx.enter_context(tc.tile_pool(name="xt", bufs=1))
        psS = ctx.enter_context(tc.tile_pool(name="psS", bufs=2, space="PSUM"))
        psO = ctx.enter_context(tc.tile_pool(name="psO", bufs=1, space="PSUM"))
        psX = ctx.enter_context(tc.tile_pool(name="psX", bufs=2, space="PSUM"))

        mask_t = [cp.tile([128, 1024], BF16, tag=f"m{i}", name=f"m{i}")
                  for i in range(NQB)]
        ones32 = cp.tile([128, 64], F32, tag="ones32", name="ones32")
        nc.vector.memset(ones32[:], 1.0)
        # preload the exp ACT table set while the input DMAs stream in
        warm = cp.tile([1, 8], F32, tag="warm", name="warm")
        nc.scalar.activation(warm[:], ones32[0:1, 0:8], EXP)
        ones_t = cp.tile([128, 64], BF16, tag="ones", name="ones")
        nc.vector.tensor_copy(ones_t[:], ones32[:])
        onesv = cp.tile([128, 4], F32, tag="onesv", name="onesv")
        nc.vector.memset(onesv[:], 1.0)
        # PE warm-up: dummy matmuls with no DMA deps flip the HAM clock
        # gate to 8/8 (2.4 GHz) before the real streams begin
        wmt = cp.tile([128, 512], BF16, tag="wmt", name="wmt")
        nc.vector.memset(wmt[:], 0.0)
        psW = psX.tile([128, 512], F32, tag="b", name="b")
        for i in range(20):
            # ~6us: bridges the input-DMA ramp so the PE never idles a
            # full MID window before the first projection group
            nc.tensor.matmul(psW[:], wmt[:, 0:128], wmt[:],
                             start=(i == 0), stop=(i == 19))

        # weights
        wq_t = [wtp.tile([128, 256], BF16, tag=f"wq{k}", name=f"wq{k}")
                for k in range(NDC)]
        wk_t = [wtp.tile([128, 256], BF16, tag=f"wk{k}", name=f"wk{k}")
                for k in range(NDC)]
        wv_t = [wtp.tile([128, 260], BF16, tag=f"wv{k}", name=f"wv{k}")
                for k in range(NDC)]
        wp_t = [wtp.tile([128, DM], BF16, tag=f"wp{j}", name=f"wp{j}")
                for j in range(2)]

        # persistent activations
        QT = [qkvp.tile([128, T], BF16, tag=f"qt{i}", name=f"qt{i}") for i in range(2)]
        KT = [qkvp.tile([128, T], BF16, tag=f"kt{i}", name=f"kt{i}") for i in range(2)]
        V = [qkvp.tile([128, 260], BF16, tag=f"v{t}", name=f"v{t}") for t in range(NKC)]
        # stacked O^T: feature-chunk jc holds heads (2jc, 2jc+1) on
        # partitions 0-63 / 64-127
        OTS = [qkvp.tile([128, T], BF16, tag=f"ots{j}", name=f"ots{j}")
               for j in range(2)]
        xt_t = [xtp.tile([128, T], BF16, tag=f"x{k}", name=f"x{k}")
                for k in range(NDC)]

        # DMA order matches consumption: Q needs wq + the first x column
        # halves (tbp=0), then the second halves, wk, wv, masks (attention
        # start), wp (projection).
        for k in range(NDC):
            nc.sync.dma_start(wq_t[k][:], wq[k * 128:(k + 1) * 128, :])
            nc.sync.dma_start(xt_t[k][:, 0:1024],
                              xT[k * 128:(k + 1) * 128, 0:1024])
        for k in range(NDC):
            nc.sync.dma_start(xt_t[k][:, 1024:2048],
                              xT[k * 128:(k + 1) * 128, 1024:2048])
        for k in range(NDC):
            nc.sync.dma_start(wk_t[k][:], wk[k * 128:(k + 1) * 128, :])
        for k in range(NDC):
            nc.sync.dma_start(wv_t[k][:], wv[k * 128:(k + 1) * 128, :])
        for i in range(NQB):
            nc.sync.dma_start(mask_t[i][:], msk[i])
        for j in range(2):
            nc.sync.dma_start(wp_t[j][:], wp[j])

        # ---------------- QKV projection pieces ----------------
        nqk = [0]

        def qk_group(W, OUT, fc, tbp):
            # full [128,1024] group in the scores PSUM pool (prefix only)
            ps = psS.tile([128, 1024], F32, tag="s", name="s")
            for j in (0, 1):
                tb = 2 * tbp + j
                for k in range(NDC):
                    nc.tensor.matmul(
                        ps[:, j * 512:(j + 1) * 512],
                        W[k][:, fc * 128:(fc + 1) * 128],
                        xt_t[k][:, tb * 512:(tb + 1) * 512],
                        start=(k == 0), stop=(k == NDC - 1))
            dst = OUT[fc][:, tbp * 1024:(tbp + 1) * 1024]
            if nqk[0] % 2 == 0:
                nc.scalar.copy(dst, ps[:])
            else:
                nc.vector.tensor_copy(dst, ps[:])
            nqk[0] += 1

        def v_tile(tt):
            def emit():
                ps = psX.tile([128, 260], F32, tag="b", name="b")
                for k in range(NDC):
                    nc.tensor.matmul(
                        ps[:], xt_t[k][:, tt * 128:(tt + 1) * 128], wv_t[k][:],
                        start=(k == 0), stop=(k == NDC - 1))
                if tt < 4 and tt % 2 == 0:
                    # prefix only: scalar is free before attention starts
                    nc.scalar.copy(V[tt][:], ps[:])
                else:
                    nc.vector.tensor_copy(V[tt][:], ps[:])
                nc.vector.tensor_copy(
                    V[tt].rearrange("p (h c) -> p h c", c=65)[:, :, 64:65],
                    onesv[:].rearrange("p (h c) -> p h c", c=1))
            return emit

        def pe_filler(n):
            # dependency-free matmuls that keep the HAM activity window
            # busy across short DMA/semaphore stalls (idle windows throttle
            # the PE clock to 1.2 GHz for ~10us)
            psd = psO.tile([65, 512], F32, tag="o0", name="o0")
            for i in range(n):
                nc.tensor.matmul(psd[:], wmt[:, 0:65], wmt[:],
                                 start=(i == 0), stop=(i == n - 1))

        # phase-B prefix: the full Q/K projections (heavy 16-MM groups do
        # not interleave well with the exp-paced attention stream) plus the
        # first four V tiles; V tiles 4-15 are light enough to inject.
        ng = 0
        for W, OUT in ((wq_t, QT), (wk_t, KT)):
            for tbp in range(2):
                for fc in range(2):
                    qk_group(W, OUT, fc, tbp)
                    ng += 1
                    if 1 <= ng <= 5:
                        pe_filler(3)
        for tt in range(4):
            v_tile(tt)()

        # ---------------- attention ----------------
        ptp = ctx.enter_context(tc.tile_pool(name="pt", bufs=5))
        rcp = ctx.enter_context(tc.tile_pool(name="rcp", bufs=2))

        def make_norm_steps(hp, dds, ou_all):
            def norm_step(h, qb, fast=False):
                def emit():
                    hh = h % 2
                    dd = dds[(h, qb)]
                    psb = psX.tile([64, 512], F32, tag="b", name="b")
                    nc.tensor.matmul(
                        psb[:], ones_t[0:1, 0:64], dd[0:1, :],
                        start=True, stop=True)
                    if hh == 0:
                        nc.vector.tensor_tensor(
                            OTS[hp][0:64, qb * 512:(qb + 1) * 512],
                            ou_all[(h, qb)][0:64, :], psb[:], AL.mult)
                    else:
                        # odd head: normalize to a bounce tile, DMA to
                        # partitions 64-127 of the stacked O^T; in the
                        # drain tail use the scalar engine's DGE queue so
                        # it never waits behind bulk y-tile DMAs
                        ob = rcp.tile([64, 512], BF16, tag="ob",
                                      name="ob", bufs=2)
                        nc.vector.tensor_tensor(
                            ob[:], ou_all[(h, qb)][0:64, :], psb[:],
                            AL.mult)
                        nc.sync.dma_start(
                            OTS[hp][64:128, qb * 512:(qb + 1) * 512],
                            ob[:])
                return emit
            return norm_step

        def proj_tile(tt, mb, ybp, on_scalar=False):
            psy = psX.tile([128, 512], F32, tag="b", name="yps")
            for jc in range(2):
                nc.tensor.matmul(
                    psy[:],
                    OTS[jc][:, tt * 128:(tt + 1) * 128],
                    wp_t[jc][:, mb * 512:(mb + 1) * 512],
                    start=(jc == 0), stop=(jc == 1))
            yt = ybp.tile([128, 512], BF16, tag="yt", name="yt")
            if on_scalar:
                nc.scalar.copy(yt[:], psy[:])
            else:
                nc.vector.tensor_copy(yt[:], psy[:])
            nc.sync.dma_start(
                y[tt * 128:(tt + 1) * 128, mb * 512:(mb + 1) * 512],
                yt[:])

        ybp = ctx.enter_context(tc.tile_pool(name="yb", bufs=3))
        # leftover V-projection tiles, interleaved into hp0's attention
        inject = [v_tile(t) for t in range(4, NKC)]

        for hp in range(HPC // 2):
            fc = hp
            heads = (2 * hp, 2 * hp + 1)
            dds = {}
            ou_all = {}
            norm_step = make_norm_steps(hp, dds, ou_all)
            for qb in range(NQB):
                if hp == 0 and qb == NQB - 1:
                    # hp0's qb0-2 normalize work runs during its own qb3
                    inject += [norm_step(h, q)
                               for q in range(3) for h in heads]
                nkc = 4 * (qb + 1)
                # hp0 paces the injected V tiles just-in-time (4 per query
                # block) so they fill PE slack instead of starving exp;
                # hp1 and hp0-qb3 drain greedily
                npop = 4 if (hp == 0 and qb in (1, 2)) else \
                    0 if (hp == 0 and qb == 0) else 999
                pso = {h: psO.tile([65, 512], F32, tag=f"o{h % 2}",
                                   name=f"o{h % 2}") for h in heads}
                ptq = {}
                for kc in range(nkc + LAG):
                    if kc < nkc:
                        # both heads' S^T for this key chunk in one PSUM
                        # tile; the two K=64 matmuls alternate PE row
                        # groups and run concurrently
                        pss = psS.tile([128, 1024], F32, tag="s", name="s")
                        for h in heads:
                            po = 64 * (h % 2)
                            nc.tensor.matmul(
                                pss[:, po * 8:po * 8 + 512],
                                KT[fc][po:po + 64, kc * 128:(kc + 1) * 128],
                                QT[fc][po:po + 64, qb * 512:(qb + 1) * 512],
                                start=True, stop=True)
                        pt = ptp.tile([128, 1024], BF16, tag="pt", name="pt")
                        nc.scalar.activation(pt[:], pss[:], EXP)
                        if kc >= 4 * qb:  # diagonal chunk -> causal mask
                            # only cols < 128*(t+1) can be zeroed (beyond
                            # that the causal mask is all-ones), so slice
                            # the multiply to the affected prefix per head
                            t = kc - 4 * qb
                            w = 128 * (t + 1)
                            if w >= 512:
                                nc.vector.tensor_tensor(
                                    pt[:], pt[:], mask_t[t][:], AL.mult)
                            else:
                                pts = pt.rearrange(
                                    "p (g c) -> p g c", c=512)[:, :, 0:w]
                                nc.vector.tensor_tensor(
                                    pts, pts,
                                    mask_t[t].rearrange(
                                        "p (g c) -> p g c", c=512)[:, :, 0:w],
                                    AL.mult)
                        ptq[kc] = pt
                    kcp = kc - LAG
                    if kcp >= 0 and kcp in ptq:
                        ptv = ptq.pop(kcp)
                        for h in heads:
                            po = 64 * (h % 2)
                            nc.tensor.matmul(
                                pso[h][:],
                                V[kcp][:, 65 * h:65 * h + 65],
                                ptv[:, po * 8:po * 8 + 512],
                                start=(kcp == 0),
                                stop=(kcp == nkc - 1))
                    if kcp >= 0 and inject and npop > 0:
                        npop -= 1
                        inject.pop(0)()
                        if len(inject) > 10 and (hp == 1 or qb == NQB - 1):
                            inject.pop(0)()
                for h in heads:
                    # O rows to SBUF; denominator row DMA'd from PSUM to a
                    # partition-0 tile (DVE ops are lane-wise and cannot
                    # shift partitions; the DMA can)
                    ou = rcp.tile([65, 512], F32,
                                  tag=f"ou{hp}_{h % 2}_{qb}",
                                  name=f"ou{hp}_{h % 2}_{qb}", bufs=1)
                    if hp == 1 and qb == 3:
                        # tail: scalar is idle (exp done) and the vector
                        # queue is full of projection copies
                        nc.scalar.copy(ou[:], pso[h][:])
                    else:
                        nc.vector.tensor_copy(ou[:], pso[h][:])
                    den = rcp.tile([1, 512], F32, tag="den",
                                   name="den", bufs=2)
                    nc.sync.dma_start(den[:], ou[64:65, :])
                    dhr = rcp.tile([1, 512], F32, tag="dhr",
                                   name="dhr", bufs=2)
                    nc.vector.reciprocal_approx_fast(dhr[:], den[:])
                    dd = rcp.tile([1, 512], BF16,
                                  tag=f"dd{hp}_{h % 2}_{qb}",
                                  name=f"dd{hp}_{h % 2}_{qb}", bufs=1)
                    nc.vector.tensor_copy(dd[:], dhr[:])
                    dds[(h, qb)] = dd
                    ou_all[(h, qb)] = ou
                if hp == 0 and qb == NQB - 1:
                    inject += [norm_step(h, 3) for h in heads]
                if hp == 1:
                    # each query block's normalize + projection batch is
                    # ready as soon as that block's attention finishes;
                    # emitting per-qb keeps every block's slots evenly fed
                    last = qb == NQB - 1
                    inject += [norm_step(h, qb, fast=last) for h in heads]
                    inject += [(lambda t=t, m=m, s=last:
                                proj_tile(t, m, ybp,
                                          on_scalar=s and (t + m) % 2 == 0))
                               for t in range(4 * qb, 4 * qb + 4)
                               for m in range(2)]
        # drain any remaining injected steps (tail of the kernel); filler
        # matmuls bridge the denominator/normalize dependency chain so the
        # PE clock stays at 2.4 GHz for the final projection tiles
        pe_filler(14)
        ndrain = 0
        while inject:
            inject.pop(0)()
            ndrain += 1
            if ndrain == 2:
                pe_filler(6)
        ctx.close()

    nc.finalize()
    return nc


def make_masks():
    """[NQB, 128, 1024]: mask for diagonal chunk offset t, duplicated for the
    two heads (cols 0-511 and 512-1023 identical).
    keep iff query_in_block >= key_in_chunk + 128*t."""
    m = np.zeros((NQB, 128, 1024), dtype=np.float32)
    f = np.arange(512)
    p = np.arange(128)
    for t in range(NQB):
        pat = (f[None, :] >= p[:, None] + 128 * t).astype(np.float32)
        m[t][:, 0:512] = pat
        m[t][:, 512:1024] = pat
    return m


def shard_inputs(x, Wqkv, bqkv, Wproj):
    x = np.asarray(x, dtype=np.float32)
    Wqkv = np.asarray(Wqkv, dtype=np.float32)
    bqkv = np.asarray(bqkv, dtype=np.float32)
    Wproj = np.asarray(Wproj, dtype=np.float32)
    assert not np.any(bqkv[0:2048]), \
        "nonzero q/k bias not supported by the fast kernel"
    bf = ml_dtypes.bfloat16
    masks = make_masks().astype(bf)
    in_maps = []
    for c in range(8):
        b, g = c // 4, c % 4
        cs = slice(256 * g, 256 * g + 256)
        wq_ = np.ascontiguousarray(Wqkv[:, 0:1024][:, cs]) / 8.0
        wk_ = np.ascontiguousarray(Wqkv[:, 1024:2048][:, cs])
        wv_src = Wqkv[:, 2048:3072][:, cs]
        wv_ = np.zeros((DM, 260), dtype=np.float32)
        for h in range(4):
            wv_[:, 65 * h:65 * h + 64] = wv_src[:, 64 * h:64 * h + 64]
        wp_ = np.ascontiguousarray(
            Wproj[256 * g:256 * g + 256, :].reshape(2, 128, DM))
        in_maps.append({
            "xT": np.ascontiguousarray(x[b].T).astype(bf),
            "wq": wq_.astype(bf), "wk": wk_.astype(bf),
            "wv": wv_.astype(bf), "wp": wp_.astype(bf), "msk": masks,
        })
    return in_maps


def combine_outputs(results, Wqkv, bqkv, Wproj, bproj):
    bqkv = np.asarray(bqkv, dtype=np.float32)
    Wproj = np.asarray(Wproj, dtype=np.float32)
    bproj = np.asarray(bproj, dtype=np.float32)
    bv_term = bqkv[2048:3072] @ Wproj
    out = np.zeros((2, T, DM), dtype=np.float32)
    for c in range(8):
        out[c // 4] += results[c]["y"].astype(np.float32)
    out += (bv_term + bproj)[None, None, :]
    return out


_NC_CACHE = []


def _numpy_fallback(x, Wqkv, bqkv, Wproj, bproj):
    # exact-but-slow path for inputs the device kernel does not support
    b, t, dm = x.shape
    h, d = 16, 64
    qkv = x @ Wqkv + bqkv
    q, k, v = np.split(qkv, 3, axis=-1)
    q = q.reshape(b, t, h, d).transpose(0, 2, 1, 3)
    k = k.reshape(b, t, h, d).transpose(0, 2, 1, 3)
    v = v.reshape(b, t, h, d).transpose(0, 2, 1, 3)
    att = np.einsum('bhqd,bhkd->bhqk', q, k) / np.sqrt(np.float32(d))
    causal = np.tril(np.ones((t, t), dtype=bool))
    att = np.where(causal[None, None], att, -np.inf)
    att = att - att.max(axis=-1, keepdims=True)
    e = np.exp(att)
    p = e / e.sum(axis=-1, keepdims=True)
    out = np.einsum('bhqk,bhkd->bhqd', p, v)
    out = out.transpose(0, 2, 1, 3).reshape(b, t, dm)
    return (out @ Wproj + bproj).astype(np.float32)


def kernel(x, Wqkv, bqkv, Wproj, bproj):
    x = np.asarray(x, dtype=np.float32)
    Wqkv = np.asarray(Wqkv, dtype=np.float32)
    bqkv = np.asarray(bqkv, dtype=np.float32)
    Wproj = np.asarray(Wproj, dtype=np.float32)
    bproj = np.asarray(bproj, dtype=np.float32)
    if np.any(bqkv[0:2048]):
        # nonzero q/k bias falls outside the fused device kernel's contract
        return _numpy_fallback(x, Wqkv, bqkv, Wproj, bproj)
    from concourse.bass_utils import run_bass_kernel_spmd
    if not _NC_CACHE:
        _NC_CACHE.append(build_nc())
    nc = _NC_CACHE[0]
    in_maps = shard_inputs(x, Wqkv, bqkv, Wproj)
    res = run_bass_kernel_spmd(nc, in_maps, core_ids=list(range(8)))
    return combine_outputs(res.results, Wqkv, bqkv, Wproj, bproj)
